# revision 10
# baseline (speedup 1.0000x reference)
"""Trainium2 Bass kernel for nn_DKT_89034672046889 (DKT-style recurrent net).

Strategy: data-parallel over batch across 8 NeuronCores (128 rows each).
On-device layout is feature-major ([feature, t*128+b]); host does
weight-only precompute (sigmoid tables, answer-embedding folds, gate-folded
recurrent weights) and input layout transforms (transpose/cast/shard).
Both scans are expressed with sigmoid-only activations:
  scan1 state v_t = sigma(2*a_t)        (u_t = tanh(a_t) = 2 v_t - 1)
  scan2 state p_t = (1+h_t)/2 in [0,1]  (p' = (1-z) p + z w)
so the ACT engine never switches function tables.
"""

import sys

for _p in ("/root/.axon_site/_ro/trn_rl_repo", "/opt/trn_rl_repo"):
    if _p not in sys.path:
        sys.path.append(_p)

import numpy as np
import ml_dtypes

import concourse.bacc as bacc
import concourse.mybir as mybir
import concourse.tile as tile
from concourse.bass import IndirectOffsetOnAxis
from concourse.bass_utils import run_bass_kernel_spmd

BF = mybir.dt.bfloat16
F32 = mybir.dt.float32
I32 = mybir.dt.int32

B, T, H, EMB = 1024, 39, 123, 256
NCORES = 8
BL = B // NCORES            # 128 batch rows per core
NT = T * BL                 # 4992 columns, t-major (n = t*128 + b)
GSZ = 512

_bf16 = ml_dtypes.bfloat16


def _sigmoid(x):
    return 1.0 / (1.0 + np.exp(-x))


def _groups():
    out = []
    c = 0
    while c < NT:
        s = min(GSZ, NT - c)
        out.append((c, s))
        c += s
    return out


def build_nc(dbg=False):
    nc = bacc.Bacc(None, target_bir_lowering=False, debug=False)

    dt = nc.dram_tensor
    qmT_d = dt("qmT", [124, NT], BF, kind="ExternalInput")       # row 123 = ones
    qmnT_d = dt("qmnT", [123, NT], BF, kind="ExternalInput")
    rrow_d = dt("rrow", [1, NT], BF, kind="ExternalInput")
    sid_d = dt("sid", [BL, 1], I32, kind="ExternalInput")
    eid_d = dt("eid", [BL, T], I32, kind="ExternalInput")
    qnx_d = dt("qnx", [BL, T], I32, kind="ExternalInput")
    stab_d = dt("stab", [100001, H], F32, kind="ExternalInput")
    kdtab_d = dt("kdtab", [50001, H + 1], F32, kind="ExternalInput")
    e3tab_d = dt("e3tab", [50001, EMB], F32, kind="ExternalInput")
    Lzz_d = dt("Lzz", [124, H], BF, kind="ExternalInput")
    Lxp_d = dt("Lxp", [125, H], BF, kind="ExternalInput")
    A1_d = dt("A1", [124, T * H], BF, kind="ExternalInput")
    Lz_d = dt("Lz", [125, H], BF, kind="ExternalInput")
    Lh2_d = dt("Lh2", [125, H], BF, kind="ExternalInput")
    Az_d = dt("Az", [H, H], BF, kind="ExternalInput")
    Ah_d = dt("Ah", [H, H], BF, kind="ExternalInput")
    o1a_d = dt("o1a", [H, EMB], BF, kind="ExternalInput")
    o1c_d = dt("o1c", [H, EMB], BF, kind="ExternalInput")
    p2T_d = dt("p2T", [128, 2 * H], BF, kind="ExternalInput")
    p3g_d = dt("p3g", [124, 1], BF, kind="ExternalInput")
    GA_d = dt("GA", [H, T], F32, kind="ExternalInput")
    GB_d = dt("GB", [H, T], F32, kind="ExternalInput")
    o1b_d = dt("o1b", [128, 2], F32, kind="ExternalInput")
    p2b_d = dt("p2b", [H, 1], F32, kind="ExternalInput")
    dtv_d = dt("dtv", [H, 1], F32, kind="ExternalInput")
    e123_d = dt("e123", [124, H], BF, kind="ExternalInput")
    ident_d = dt("ident", [128, 128], F32, kind="ExternalInput")

    out_d = dt("out", [BL, T], F32, kind="ExternalOutput")
    dbg_d = {}
    if dbg:
        for nm, shp in (("dRHS1", [125, NT]), ("dV", [124, (T + 1) * BL]),
                        ("dRHS2", [125, NT]), ("dP", [H, (T + 1) * BL]),
                        ("dO1", [128, 2 * NT]), ("dO2", [124, NT]),
                        ("dDKT", [124, NT]), ("dqmDT", [H, NT]),
                        ("dDKQ", [H, NT])):
            dbg_d[nm] = dt(nm, shp, BF, kind="ExternalOutput")
        dbg_d["dSPT"] = dt("dSPT", [H, BL], F32, kind="ExternalOutput")

    groups = _groups()

    with tile.TileContext(nc) as tc:
        with (
            tc.tile_pool(name="per", bufs=1) as per,
            tc.tile_pool(name="gat", bufs=12) as gat,
            tc.tile_pool(name="tmp", bufs=4) as tmp,
            tc.tile_pool(name="zw", bufs=3) as zwp,
            tc.tile_pool(name="psA", bufs=2, space="PSUM") as psA,
            tc.tile_pool(name="psPT", bufs=2, space="PSUM") as psPT,
            tc.tile_pool(name="psT", bufs=1, space="PSUM") as psT,
            tc.tile_pool(name="psO1", bufs=1, space="PSUM") as psO1,
            tc.tile_pool(name="psO2", bufs=1, space="PSUM") as psO2,
            tc.tile_pool(name="psO3", bufs=1, space="PSUM") as psO3,
        ):
            sync, gp, ve, se, te = nc.sync, nc.gpsimd, nc.vector, nc.scalar, nc.tensor
            SIG = mybir.ActivationFunctionType.Sigmoid
            MUL = mybir.AluOpType.mult
            ADD = mybir.AluOpType.add
            SUB = mybir.AluOpType.subtract

            # ---- persistent SBUF tiles ----
            qmT = per.tile([124, NT], BF)
            qmnT = per.tile([123, NT], BF)
            RHS1 = per.tile([125, NT], BF)
            RHS2 = per.tile([125, NT], BF)
            V = per.tile([124, (T + 1) * BL], BF)
            P = per.tile([H, (T + 1) * BL], BF)
            DKT = per.tile([124, NT], BF)
            qmDT = per.tile([H, NT], BF)
            DKQ = per.tile([H, NT], BF)
            O1 = per.tile([128, 2 * NT], BF)
            O2 = per.tile([124, NT], BF)
            sidt = per.tile([BL, 1], I32)
            eidt = per.tile([BL, T], I32)
            qnxt = per.tile([BL, T], I32)
            Lzz = per.tile([124, H], BF)
            Lxp = per.tile([125, H], BF)
            A1 = per.tile([124, T * H], BF)
            Lz = per.tile([125, H], BF)
            Lh2 = per.tile([125, H], BF)
            Az = per.tile([H, H], BF)
            Ah = per.tile([H, H], BF)
            o1a = per.tile([H, EMB], BF)
            o1c = per.tile([H, EMB], BF)
            p2T = per.tile([128, 2 * H], BF)
            p3g = per.tile([124, 1], BF)
            GA = per.tile([H, T], F32)
            GB = per.tile([H, T], F32)
            o1b = per.tile([128, 2], F32)
            p2b = per.tile([H, 1], F32)
            dtv = per.tile([H, 1], F32)
            e123 = per.tile([124, H], BF)
            ident = per.tile([128, 128], F32)
            SPT = per.tile([H, BL], F32)
            SPD = per.tile([H, BL], BF)
            SPT4 = per.tile([H, GSZ], F32)
            OUTt = per.tile([BL, T], F32)

            # ---- init (before any DMA into these tiles) ----
            ve.memset(RHS1[:], 1.0)
            ve.memset(RHS2[:], 1.0)
            ve.memset(V[:], 1.0)
            ve.memset(V[0:H, 0:BL], 0.0)
            ve.memset(P[:], 0.5)
            ve.memset(O2[:], 1.0)

            # ---- loads ----
            sync.dma_start(out=sidt[:], in_=sid_d[:])
            sync.dma_start(out=eidt[:], in_=eid_d[:])
            sync.dma_start(out=qnxt[:], in_=qnx_d[:])
            for dst, src in (
                (Lzz, Lzz_d), (Lxp, Lxp_d), (A1, A1_d), (Lz, Lz_d),
                (Lh2, Lh2_d), (Az, Az_d), (Ah, Ah_d), (o1a, o1a_d),
                (o1c, o1c_d), (p2T, p2T_d), (p3g, p3g_d), (GA, GA_d),
                (GB, GB_d), (o1b, o1b_d), (p2b, p2b_d), (dtv, dtv_d),
                (e123, e123_d), (ident, ident_d),
            ):
                sync.dma_start(out=dst[:], in_=src[:])
            sync.dma_start(out=qmT[:], in_=qmT_d[:])
            sync.dma_start(out=qmnT[:], in_=qmnT_d[:])
            sync.dma_start(out=RHS1[123:124, :], in_=rrow_d[:])
            sync.dma_start(out=RHS2[123:124, :], in_=rrow_d[:])

            # ---- student profile ----
            sg = gat.tile([BL, H], F32, tag="sg", bufs=1)
            gp.indirect_dma_start(
                out=sg[:], out_offset=None, in_=stab_d[:],
                in_offset=IndirectOffsetOnAxis(ap=sidt[:, 0:1], axis=0))
            pT = psT.tile([124, 128], F32, space="PSUM", tag="tp")
            te.transpose(out=pT[0:H, :], in_=sg[:], identity=ident[:])
            ve.tensor_copy(out=SPT[:], in_=pT[0:H, :])
            ve.tensor_scalar(out=SPD[:], in0=SPT[:], scalar1=dtv[:], scalar2=None,
                             op0=MUL)
            for i in range(4):
                ve.tensor_copy(out=SPT4[:, i * BL:(i + 1) * BL], in_=SPT[:])

            # ---- concept_ab + x1a -> RHS1 rows 0:123 ----
            for (c0, csz) in groups:
                pa = psA.tile([H, GSZ], F32, space="PSUM", tag="a")
                te.matmul(out=pa[:, 0:csz], lhsT=Lzz[:], rhs=qmT[:, c0:c0 + csz],
                          start=True, stop=True)
                ve.tensor_tensor(out=RHS1[0:H, c0:c0 + csz], in0=pa[:, 0:csz],
                                 in1=SPT4[:, 0:csz], op=MUL)

            # ---- pipelined: kd/e3 gathers + DKT + per-group qmDT/DKQ + scan1 + middle ----
            e3tiles = [None] * T
            ps1 = [None]
            for t in range(T + 4):
                if t < T:
                    # gathers paced with the scan
                    kg = gat.tile([BL, H + 1], F32, tag="kg")
                    gp.indirect_dma_start(
                        out=kg[:], out_offset=None, in_=kdtab_d[:],
                        in_offset=IndirectOffsetOnAxis(ap=eidt[:, t:t + 1], axis=0))
                    eg = gat.tile([BL, EMB], F32, tag="eg")
                    gp.indirect_dma_start(
                        out=eg[:], out_offset=None, in_=e3tab_d[:],
                        in_offset=IndirectOffsetOnAxis(ap=qnxt[:, t:t + 1], axis=0))
                    e3tiles[t] = eg
                    pk = psT.tile([124, 128], F32, space="PSUM", tag="tp")
                    te.transpose(out=pk[:], in_=kg[:], identity=ident[:])
                    ve.tensor_copy(out=DKT[:, t * BL:(t + 1) * BL], in_=pk[:])
                    if t % 4 == 3 or t == T - 1:
                        g = t // 4
                        c0, csz = groups[g]
                        pd = psA.tile([H, GSZ], F32, space="PSUM", tag="a")
                        te.matmul(out=pd[:, 0:csz], lhsT=e123[:],
                                  rhs=DKT[:, c0:c0 + csz], start=True, stop=True)
                        ve.tensor_tensor(out=qmDT[:, c0:c0 + csz], in0=pd[:, 0:csz],
                                         in1=qmT[0:H, c0:c0 + csz], op=MUL)
                        ve.tensor_tensor(out=DKQ[:, c0:c0 + csz],
                                         in0=DKT[0:H, c0:c0 + csz],
                                         in1=qmT[0:H, c0:c0 + csz], op=MUL)
                    # scan1 tick t
                    gi, off = divmod(t, 4)
                    if off == 0:
                        c0 = gi * GSZ
                        csz = min(GSZ, NT - c0)
                        ps1[0] = psA.tile([H, GSZ], F32, space="PSUM", tag="a", name="ps1g")
                        te.matmul(out=ps1[0][:, 0:csz], lhsT=Lxp[:],
                                  rhs=RHS1[:, c0:c0 + csz], start=True, stop=True)
                    sl = slice(off * BL, (off + 1) * BL)
                    te.matmul(out=ps1[0][:, sl], lhsT=A1[:, t * H:(t + 1) * H],
                              rhs=V[:, t * BL:(t + 1) * BL], start=False, stop=True)
                    se.activation(out=V[0:H, (t + 1) * BL:(t + 2) * BL],
                                  in_=ps1[0][:, sl], func=SIG)
                # middle, lagged 4 ticks
                tm = t - 4
                if 0 <= tm < T:
                    m1 = tmp.tile([H, BL], BF, tag="m1")
                    ve.tensor_scalar(out=m1[:],
                                     in0=V[0:H, (tm + 1) * BL:(tm + 2) * BL],
                                     scalar1=GA[:, tm:tm + 1],
                                     scalar2=GB[:, tm:tm + 1], op0=MUL, op1=ADD)
                    m2 = tmp.tile([H, BL], BF, tag="m2")
                    ve.tensor_tensor(out=m2[:], in0=m1[:], in1=SPD[:], op=ADD)
                    m3 = tmp.tile([H, BL], BF, tag="m3")
                    ve.tensor_tensor(out=m3[:], in0=m2[:],
                                     in1=qmDT[:, tm * BL:(tm + 1) * BL], op=MUL)
                    ve.tensor_tensor(out=RHS2[0:H, tm * BL:(tm + 1) * BL], in0=m3[:],
                                     in1=DKQ[:, tm * BL:(tm + 1) * BL], op=SUB)

            # ---- scan2 + o1/o2/o3 pipelined ----
            for t in range(T + 8):
                if t < T:
                    pt = psPT.tile([H, 256], F32, space="PSUM", tag="pt")
                    te.matmul(out=pt[:, 0:128], lhsT=Lz[:],
                              rhs=RHS2[:, t * BL:(t + 1) * BL], start=True, stop=True)
                    te.matmul(out=pt[:, 128:256], lhsT=Lh2[:],
                              rhs=RHS2[:, t * BL:(t + 1) * BL], start=False, stop=True)
                    te.matmul(out=pt[:, 0:128], lhsT=Az[:],
                              rhs=P[:, t * BL:(t + 1) * BL], start=False, stop=True)
                    te.matmul(out=pt[:, 128:256], lhsT=Ah[:],
                              rhs=P[:, t * BL:(t + 1) * BL], start=False, stop=True)
                    zw = zwp.tile([H, 256], BF, tag="zw")
                    se.activation(out=zw[:], in_=pt[:], func=SIG)
                    d1 = tmp.tile([H, BL], BF, tag="d1")
                    ve.tensor_tensor(out=d1[:], in0=zw[:, 128:256],
                                     in1=P[:, t * BL:(t + 1) * BL], op=SUB)
                    d2 = tmp.tile([H, BL], BF, tag="d2")
                    ve.tensor_tensor(out=d2[:], in0=zw[:, 0:128], in1=d1[:], op=MUL)
                    ve.tensor_tensor(out=P[:, (t + 1) * BL:(t + 2) * BL],
                                     in0=P[:, t * BL:(t + 1) * BL], in1=d2[:], op=ADD)
                # o1 for group g once its last tick (4g+3) is done, lag 1
                if t >= 4 and (t - 4) % 4 == 0:
                    g = (t - 4) // 4
                    if g < len(groups):
                        c0, csz = groups[g]
                        ts = range(g * 4, min(g * 4 + 4, T))
                        for m in range(2):
                            po = psO1.tile([128, GSZ], F32, space="PSUM", tag="o1")
                            for i, tt in enumerate(ts):
                                te.matmul(out=po[:, i * BL:(i + 1) * BL],
                                          lhsT=e3tiles[tt][:, m * 128:(m + 1) * 128],
                                          rhs=ident[:], is_transpose=True,
                                          start=(i == 0), stop=True)
                            te.matmul(out=po[:, 0:csz],
                                      lhsT=o1a[:, m * 128:(m + 1) * 128],
                                      rhs=P[:, BL + c0:BL + c0 + csz],
                                      start=False, stop=True)
                            te.matmul(out=po[:, 0:csz],
                                      lhsT=o1c[:, m * 128:(m + 1) * 128],
                                      rhs=qmnT[:, c0:c0 + csz], start=False, stop=True)
                            se.activation(out=O1[:, m * NT + c0:m * NT + c0 + csz],
                                          in_=po[:, 0:csz], func=SIG,
                                          bias=o1b[:, m:m + 1])
                # o2 for group g, lag 2 groups
                if t >= 8 and (t - 8) % 4 == 0:
                    g = (t - 8) // 4
                    if g < len(groups):
                        c0, csz = groups[g]
                        p2 = psO2.tile([H, GSZ], F32, space="PSUM", tag="o2")
                        te.matmul(out=p2[:, 0:csz], lhsT=p2T[:, 0:H],
                                  rhs=O1[:, c0:c0 + csz], start=True, stop=True)
                        te.matmul(out=p2[:, 0:csz], lhsT=p2T[:, H:2 * H],
                                  rhs=O1[:, NT + c0:NT + c0 + csz],
                                  start=False, stop=True)
                        se.activation(out=O2[0:H, c0:c0 + csz], in_=p2[:, 0:csz],
                                      func=SIG, bias=p2b[:])

            # o1/o2 for the last groups (if pipeline lags left any)
            # (covered: T+8 = 47 ticks -> o1 g<=10, o2 g<=9 -> all 10 groups emitted)

            # ---- o3 ----
            p3 = psO3.tile([128, T], F32, space="PSUM", tag="o3")
            for t in range(T):
                te.matmul(out=p3[:, t:t + 1], lhsT=O2[:, t * BL:(t + 1) * BL],
                          rhs=p3g[:], start=(t == 0), stop=True)
            se.activation(out=OUTt[:], in_=p3[:], func=SIG)
            sync.dma_start(out=out_d[:], in_=OUTt[:])
            if dbg:
                for nm, src_tile in (("dRHS1", RHS1), ("dV", V), ("dRHS2", RHS2),
                                     ("dP", P), ("dO1", O1), ("dO2", O2),
                                     ("dDKT", DKT), ("dqmDT", qmDT),
                                     ("dDKQ", DKQ)):
                    sync.dma_start(out=dbg_d[nm][:], in_=src_tile[:])
                sync.dma_start(out=dbg_d["dSPT"][:], in_=SPT[:])

    nc.finalize()
    return nc


def host_prep(inputs):
    """Weight-only precompute + input layout transforms. Returns in_maps."""
    f = lambda k: np.asarray(inputs[k], np.float32)
    ii = lambda k: np.asarray(inputs[k]).astype(np.int32)

    d_t = float(f("d_t")[0])
    d_e = float(f("d_e")[0])
    W_ih, b_ih = f("W_ih"), f("b_ih")
    W_hh, b_hh = f("W_hh"), f("b_hh")
    W_z, b_z = f("W_z"), f("b_z")
    W_h, b_h = f("W_h"), f("b_h")
    answer_W = f("answer_W")
    zz_W, zz_b = f("zz_W"), f("zz_b")
    p1_W, p1_b = f("p1_W"), f("p1_b")
    p2_W, p2_b = f("p2_W"), f("p2_b")
    p3_W, p3_b = f("p3_W"), f("p3_b")
    W_tg, b_tg = f("W_tg"), f("b_tg")

    tvec = np.arange(T, dtype=np.float32)[:, None]
    G = _sigmoid(tvec * W_tg[:, 0][None, :] + b_tg)          # [T,123]

    stab = _sigmoid(f("student_W")).astype(np.float32)
    D_tab = _sigmoid(f("e_disc_W")[:, 0]) * d_e
    kdtab = np.concatenate(
        [_sigmoid(f("k_diff_W")) * D_tab[:, None], D_tab[:, None]], 1
    ).astype(np.float32)
    e3tab = (f("emb_problem") @ p1_W[:, 123:379].T).astype(np.float32)

    def fold(Wm, bias):
        ap = answer_W @ Wm[:, 123:379].T
        return ap[0] + bias, ap[1] - ap[0]
    c0_ih, dl_ih = fold(W_ih, b_ih)
    c0_z, dl_z = fold(W_z, b_z)
    c0_h, dl_h = fold(W_h, b_h)
    Wz_h = W_z[:, 379:502]
    Wh_h = W_h[:, 379:502]

    Lzz = np.concatenate([zz_W.T, zz_b[None]], 0)
    Lxp = np.concatenate([2 * W_ih[:, :123].T, 2 * dl_ih[None], 2 * c0_ih[None]], 0)
    A1_flat = np.zeros((124, T * H), np.float32)
    A1_flat[123, 0:H] = 2 * b_hh
    for t in range(1, T):
        g = G[t - 1]
        A1_flat[:123, t * H:(t + 1) * H] = 4.0 * g[:, None] * W_hh.T
        A1_flat[123, t * H:(t + 1) * H] = 2 * b_hh - 2.0 * (W_hh @ g)
    Lz = np.concatenate([W_z[:, :123].T, dl_z[None], (c0_z - Wz_h.sum(1))[None]], 0)
    Lh2 = np.concatenate(
        [2 * W_h[:, :123].T, 2 * dl_h[None], (2 * c0_h - 2 * Wh_h.sum(1))[None]], 0)
    Az = 2.0 * Wz_h.T
    Ah = 4.0 * Wh_h.T
    o1a = 2.0 * p1_W[:, :123].T
    o1c = p1_W[:, 379:502].T
    o1b = (p1_b - p1_W[:, :123].sum(1)).reshape(2, 128).T.copy()
    p2T = np.zeros((128, 2 * H), np.float32)
    p2T[:, 0:H] = p2_W.T[0:128]
    p2T[:, H:2 * H] = p2_W.T[128:256]
    p3g = np.concatenate([p3_W[0], p3_b]).reshape(124, 1)
    GA = (2.0 * (1.0 - d_t)) * G.T
    GB = (-(1.0 - d_t)) * G.T
    ident = np.eye(128, dtype=np.float32)
    dtv = np.full((H, 1), d_t, np.float32)
    e123 = np.zeros((124, H), np.float32)
    e123[123, :] = 1.0

    bf = lambda x: np.ascontiguousarray(x, np.float32).astype(_bf16)
    shared = dict(
        stab=stab, kdtab=kdtab, e3tab=e3tab,
        Lzz=bf(Lzz), Lxp=bf(Lxp), A1=bf(A1_flat), Lz=bf(Lz), Lh2=bf(Lh2),
        Az=bf(Az), Ah=bf(Ah), o1a=bf(o1a), o1c=bf(o1c), p2T=bf(p2T),
        p3g=bf(p3g), GA=GA.astype(np.float32), GB=GB.astype(np.float32),
        o1b=o1b.astype(np.float32), p2b=p2_b.reshape(H, 1).astype(np.float32),
        dtv=dtv, e123=bf(e123), ident=ident,
    )

    qm = f("q_maritx")
    qmn = f("q_maritx_next")
    r = np.asarray(inputs["r"]).astype(np.float32)
    sid = ii("s_id")
    eid = ii("e_id")
    qnx = ii("q_next")

    in_maps = []
    for c in range(NCORES):
        sl = slice(c * BL, (c + 1) * BL)
        qmTc = np.ones((124, NT), np.float32)
        qmTc[0:H] = qm[sl].transpose(2, 1, 0).reshape(H, NT)
        qmnTc = qmn[sl].transpose(2, 1, 0).reshape(H, NT)
        rrow = r[sl].T.reshape(1, NT)
        m = dict(shared)
        m.update(
            qmT=bf(qmTc), qmnT=bf(qmnTc), rrow=bf(rrow),
            sid=sid[sl].reshape(BL, 1), eid=eid[sl], qnx=qnx[sl],
        )
        in_maps.append(m)
    return in_maps


_NC_CACHE = {}


def kernel(**inputs):
    if "nc" not in _NC_CACHE:
        _NC_CACHE["nc"] = build_nc()
    nc = _NC_CACHE["nc"]
    in_maps = host_prep(inputs)
    res = run_bass_kernel_spmd(nc, in_maps, core_ids=list(range(NCORES)))
    out = np.concatenate([r["out"] for r in res.results], 0)   # [1024,39]
    return out.reshape(B, T, 1).astype(np.float32)
